# revision 1
# baseline (speedup 1.0000x reference)
"""GAT 2-layer GNN kernel for 8 Trainium2 NeuronCores.

Strategy (graph/data parallel, per the sharding hint):
  - Nodes (and their incident edges, keyed by dst) are partitioned into 8
    contiguous shards of 6250 nodes.
  - Each core computes its shard's node features h = x @ W1 plus the
    attention scalars a_src/a_dst, packs them into 256-byte table rows
    [h(bf16 x64) | a_src(f32 x8) | a_dst(f32 x8) | pad], and an AllGather
    replicates the full 50000-row table to every core (the "halo exchange" -
    with a uniformly random graph every boundary is shared).
  - Edges are laid out in ELL style: each core's dst nodes are grouped into
    blocks of 128 (degree-balanced via a lexicographic (lo-deg, hi-deg)
    sort), and an edge sits on SBUF partition = its dst's position in the
    block, chunk = its index within the dst's edge list. h[src]/a_src[src]
    rows are fetched per-edge with dma_gather (int16 indices, so the table
    is addressed from two bases: rows < 32768 and rows >= 32768); the
    block's own member rows (for a_dst[dst]) ride along as two extra
    chunks per block.
  - With dst == partition, the segment softmax-aggregate needs no one-hot
    matrix: R[e, :] = [exp(leakyrelu(a_src[src]+a_dst[dst])) * h_src | exp]
    (zeroed on padding slots via a host mask stream) and PSUM accumulates
    identity^T @ R over the block's chunks, i.e. a plain per-partition sum
    giving [numerator | denominator] per dst.
  - Layer 2 repeats the same pipeline (1 head, 40 channels) on the
    relu(out1) features (whose table rows are written in block order; the
    host translates gather indices accordingly), followed by a fused
    log_softmax. The host undoes the block permutation on the output.
"""

import os
import sys

sys.path.insert(0, "/opt/trn_rl_repo")

import numpy as np
import ml_dtypes

import concourse.bacc as bacc
import concourse.mybir as mybir
from concourse import tile
from concourse.bass_utils import run_bass_kernel_spmd
from concourse.masks import make_identity

bf16 = ml_dtypes.bfloat16

N_NODES = 50000
F_IN = 512
H1 = 8
HID = 8
D1 = H1 * HID  # 64
C2 = 40
N_CORES = 8
SHARD = N_NODES // N_CORES  # 6250
BLK = 128
NB = (SHARD + BLK - 1) // BLK  # 49 blocks per core (last has 106 dsts)
SPLIT = 32768  # int16 index range split for the gather table
SBG = 3  # blocks per gather super-group
NEG_SLOPE = 0.2
TROW = 128  # table row: 128 bf16 = 256 bytes

f32 = mybir.dt.float32
bfl = mybir.dt.bfloat16
i16 = mybir.dt.int16

_CACHE = {}


def _install_ntff_hook():
    """Provide antenv.axon_hooks if the image lacks it, driving NTFF
    profiling via the injected libaxon_pjrt.so C ABI (see trn_boot)."""
    try:
        from antenv.axon_hooks import get_axon_ntff_profile_hook  # noqa: F401
        return
    except ImportError:
        pass
    import contextlib
    import ctypes
    import types

    so_path = "/opt/axon/libaxon_pjrt.so"
    try:
        lib = ctypes.CDLL(so_path)
    except OSError:
        return
    if not hasattr(lib, "axon_start_nrt_profile"):
        return
    lib.axon_start_nrt_profile.argtypes = [ctypes.POINTER(ctypes.c_int64),
                                           ctypes.c_size_t]
    lib.axon_start_nrt_profile.restype = ctypes.c_int64
    lib.axon_stop_nrt_profile.argtypes = [ctypes.c_char_p]
    lib.axon_stop_nrt_profile.restype = ctypes.c_int64

    @contextlib.contextmanager
    def _hook(output_dir, device_ids):
        import jax
        jax.devices()
        if device_ids:
            ids = (ctypes.c_int64 * len(device_ids))(*device_ids)
            rc = lib.axon_start_nrt_profile(ids, len(device_ids))
        else:
            rc = lib.axon_start_nrt_profile(None, 0)
        if rc != 0:
            raise RuntimeError(f"axon_start_nrt_profile rc={rc}")
        try:
            yield
        finally:
            n = lib.axon_stop_nrt_profile(str(output_dir).encode())
            print(f"ntff profile: {n} file(s) written to {output_dir}")

    import antenv
    mod = types.ModuleType("antenv.axon_hooks")
    mod.get_axon_ntff_profile_hook = lambda: _hook
    mod.set_axon_ntff_profile_hook = lambda h: None
    sys.modules["antenv.axon_hooks"] = mod
    antenv.axon_hooks = mod


def _ceil(a, b):
    return (a + b - 1) // b


class LayerPlan:
    """Host-side ELL layout for one layer's edge phase (all cores)."""


def _plan_layer(src_row, dst_node, row_of=None):
    """Build the ELL plan. src_row: per-edge gather row id in the table
    (layer specific); dst_node: per-edge global dst node id; row_of[c, n]:
    global table row of node n of core c (None -> node-ordered)."""
    plan = LayerPlan()
    core = dst_node // SHARD
    local = dst_node - core * SHARD
    hi = (src_row >= SPLIT).astype(np.int64)

    # per-(core,node) lo/hi degree
    klo = np.zeros((N_CORES, SHARD), np.int64)
    khi = np.zeros((N_CORES, SHARD), np.int64)
    np.add.at(klo, (core, local), 1 - hi)
    np.add.at(khi, (core, local), hi)

    # block membership per core: lexicographic (klo desc, khi desc) sort
    # -> blocks of 128 with similar lo/hi degrees
    perm = np.full((N_CORES, NB * BLK), -1, np.int64)
    order = np.lexsort((-khi, -klo), axis=-1)
    for c in range(N_CORES):
        perm[c, :SHARD] = order[c]
    slot_of = np.zeros((N_CORES, SHARD), np.int64)
    for c in range(N_CORES):
        slot_of[c, order[c]] = np.arange(SHARD)

    # per-block chunk counts, uniform across cores
    klo_pad = np.zeros((N_CORES, NB * BLK), np.int64)
    khi_pad = np.zeros((N_CORES, NB * BLK), np.int64)
    for c in range(N_CORES):
        klo_pad[c, :SHARD] = klo[c, order[c]]
        khi_pad[c, :SHARD] = khi[c, order[c]]
    nch_lo = np.maximum(klo_pad.reshape(N_CORES, NB, BLK).max(axis=(0, 2)), 1)
    nch_hi = np.maximum(khi_pad.reshape(N_CORES, NB, BLK).max(axis=(0, 2)), 1)

    # chunk layout per super-group:
    #   [lo runs of blocks][member-lo chunk per block][hi runs][member-hi]
    ngroups = _ceil(NB, SBG)
    lo_runs = [None] * NB
    hi_runs = [None] * NB
    mlo_ch = [0] * NB
    mhi_ch = [0] * NB
    groups = []
    goff = 0
    for g in range(ngroups):
        blocks = list(range(g * SBG, min((g + 1) * SBG, NB)))
        ch = goff
        for b in blocks:
            lo_runs[b] = (ch, int(nch_lo[b]))
            ch += int(nch_lo[b])
        for b in blocks:
            mlo_ch[b] = ch
            ch += 1
        nlo_ch = ch - goff
        for b in blocks:
            hi_runs[b] = (ch, int(nch_hi[b]))
            ch += int(nch_hi[b])
        for b in blocks:
            mhi_ch[b] = ch
            ch += 1
        groups.append((blocks, goff, ch - goff, nlo_ch))
        goff = ch
    total_ch = goff

    # per-core streams
    lo_starts = np.array([r[0] for r in lo_runs], np.int64)
    hi_starts = np.array([r[0] for r in hi_runs], np.int64)
    idx_streams, mask_streams = [], []
    for c in range(N_CORES):
        sel = core == c
        e_row = src_row[sel]
        e_loc = local[sel]
        e_hi = hi[sel]
        e_slot = slot_of[c, e_loc]
        e_blk = e_slot // BLK
        e_p = e_slot % BLK
        key = e_slot * 2 + e_hi
        o = np.argsort(key, kind="stable")
        inv = np.empty_like(o)
        inv[o] = np.arange(len(o))
        e_pos = _running_count(key[o])[inv]
        e_ch = np.where(e_hi == 0, lo_starts[e_blk], hi_starts[e_blk]) + e_pos
        slots = e_ch * BLK + e_p

        idx = np.zeros(total_ch * BLK, np.int16)
        mask = np.zeros(total_ch * BLK, np.float32)
        idx[slots] = (e_row - e_hi * SPLIT).astype(np.int16)
        mask[slots] = 1.0
        for b in range(NB):
            mem = perm[c, b * BLK:(b + 1) * BLK]
            valid = mem >= 0
            if row_of is None:
                mrow = np.where(valid, mem + c * SHARD, 0)
            else:
                mrow = np.where(valid, row_of[c, mem.clip(0)], 0)
            is_lo = mrow < SPLIT
            s0 = mlo_ch[b] * BLK
            idx[s0:s0 + BLK] = np.where(valid & is_lo, mrow, 0).astype(np.int16)
            mask[s0:s0 + BLK] = (valid & is_lo).astype(np.float32)
            s1 = mhi_ch[b] * BLK
            idx[s1:s1 + BLK] = np.where(valid & ~is_lo, mrow - SPLIT,
                                        0).astype(np.int16)
            mask[s1:s1 + BLK] = (valid & ~is_lo).astype(np.float32)

        idx_w = np.tile(idx.reshape(total_ch * 8, 16).T, (8, 1)).copy()
        mask_w = mask.reshape(total_ch, BLK).T.astype(bf16).copy()
        idx_streams.append(idx_w)
        mask_streams.append(mask_w)

    plan.nch_lo = nch_lo
    plan.nch_hi = nch_hi
    plan.groups = groups
    plan.total_ch = total_ch
    plan.mlo_ch = mlo_ch
    plan.mhi_ch = mhi_ch
    plan.lo_runs = lo_runs
    plan.hi_runs = hi_runs
    plan.idx_streams = idx_streams
    plan.mask_streams = mask_streams
    plan.perm = perm
    return plan


def _running_count(k):
    """pos[i] = number of j<i with k[j]==k[i]; k is sorted."""
    n = len(k)
    if n == 0:
        return np.zeros(0, np.int64)
    starts = np.r_[0, np.flatnonzero(np.diff(k)) + 1]
    run_id = np.zeros(n, np.int64)
    run_id[starts[1:]] = 1
    run_id = np.cumsum(run_id)
    return np.arange(n) - starts[run_id]


def _prep(edge_index):
    src = np.asarray(edge_index[0], dtype=np.int64)
    dst = np.asarray(edge_index[1], dtype=np.int64)
    loops = np.arange(N_NODES, dtype=np.int64)
    src = np.concatenate([src, loops])
    dst = np.concatenate([dst, loops])

    # layer 1: table rows are node-ordered
    plan1 = _plan_layer(src, dst)

    # layer 2: table rows are block-slot-ordered per core
    s_core = src // SHARD
    s_local = src - s_core * SHARD
    slot_of1 = np.zeros((N_CORES, SHARD), np.int64)
    for c in range(N_CORES):
        slot_of1[c, plan1.perm[c, :SHARD]] = np.arange(SHARD)
    src_row2 = s_core * SHARD + slot_of1[s_core, s_local]
    row_of2 = slot_of1 + (np.arange(N_CORES) * SHARD)[:, None]
    plan2 = _plan_layer(src_row2, dst, row_of=row_of2)
    return plan1, plan2


def _build(plan1, plan2):
    nc = bacc.Bacc("TRN2", target_bir_lowering=False, debug=False,
                   num_devices=N_CORES, num_swdge_queues=2)

    NPADROWS = NB * BLK  # 6272 (last 22 cols scratch)
    xT_ext = nc.declare_dram_parameter("xT", [F_IN, NPADROWS], bfl, isOutput=False)
    w1_ext = nc.declare_dram_parameter("w1r", [128, 4 * D1], bfl, isOutput=False)
    w2_ext = nc.declare_dram_parameter("w2", [D1, C2], bfl, isOutput=False)
    a1s_ext = nc.declare_dram_parameter("a1srep", [128, D1], f32, isOutput=False)
    a1d_ext = nc.declare_dram_parameter("a1drep", [128, D1], f32, isOutput=False)
    a2s_ext = nc.declare_dram_parameter("a2srep", [128, C2], f32, isOutput=False)
    a2d_ext = nc.declare_dram_parameter("a2drep", [128, C2], f32, isOutput=False)
    b1_ext = nc.declare_dram_parameter("b1rep", [128, D1], f32, isOutput=False)
    b2_ext = nc.declare_dram_parameter("b2rep", [128, C2], f32, isOutput=False)
    idx1_ext = nc.declare_dram_parameter("idx1", [128, plan1.total_ch * 8], i16,
                                         isOutput=False)
    msk1_ext = nc.declare_dram_parameter("msk1", [128, plan1.total_ch], bfl,
                                         isOutput=False)
    idx2_ext = nc.declare_dram_parameter("idx2", [128, plan2.total_ch * 8], i16,
                                         isOutput=False)
    msk2_ext = nc.declare_dram_parameter("msk2", [128, plan2.total_ch], bfl,
                                         isOutput=False)
    out_ext = nc.declare_dram_parameter("out", [NB * BLK, C2], f32, isOutput=True)
    debug = os.environ.get("K_DEBUG", "0") == "1"
    dbg1_ext = (nc.declare_dram_parameter("dbg1", [NB * BLK, D1 + H1], f32,
                                          isOutput=True) if debug else None)

    t1_shard = nc.dram_tensor("t1_shard", [SHARD, TROW], bfl)
    t1_full = nc.dram_tensor("t1_full", [N_NODES, TROW], bfl, addr_space="Shared")
    t2_shard = nc.dram_tensor("t2_shard", [SHARD, TROW], bfl)
    t2_full = nc.dram_tensor("t2_full", [N_NODES, TROW], bfl, addr_space="Shared")

    rg = [list(range(N_CORES))]

    with tile.TileContext(nc) as tc:
        with tc.tile_pool(name="const", bufs=1) as cpool:
            ident = cpool.tile([128, 128], bfl)
            make_identity(nc, ident[:, :])
            a1s_t = cpool.tile([128, D1], f32)
            nc.sync.dma_start(out=a1s_t[:, :], in_=a1s_ext[:, :])
            a1d_t = cpool.tile([128, D1], f32)
            nc.sync.dma_start(out=a1d_t[:, :], in_=a1d_ext[:, :])
            a2s_t = cpool.tile([128, C2], f32)
            nc.sync.dma_start(out=a2s_t[:, :], in_=a2s_ext[:, :])
            a2d_t = cpool.tile([128, C2], f32)
            nc.sync.dma_start(out=a2d_t[:, :], in_=a2d_ext[:, :])
            b1_t = cpool.tile([128, D1], f32)
            nc.sync.dma_start(out=b1_t[:, :], in_=b1_ext[:, :])
            b2_t = cpool.tile([128, C2], f32)
            nc.sync.dma_start(out=b2_t[:, :], in_=b2_ext[:, :])
            w2_t = cpool.tile([D1, C2], bfl)
            nc.sync.dma_start(out=w2_t[:, :], in_=w2_ext[:, :])
            tab1_sb = cpool.tile([128, NB, TROW], bfl)
            tab2_sb = cpool.tile([128, NB, TROW], bfl)
            nc.vector.memset(tab1_sb[:, :, :], 0.0)
            nc.vector.memset(tab2_sb[:, :, :], 0.0)

            stage = int(os.environ.get("K_STAGE", "3"))

            # ---------------- Phase A/B: h1 = x @ W1, attention scalars ---
            with tc.tile_pool(name="phA", bufs=2) as apool, \
                 tc.tile_pool(name="phA_ps", bufs=2, space="PSUM") as apsum:
                w1_t = apool.tile([128, 4, D1], bfl, tag="w1")
                nc.sync.dma_start(out=w1_t[:, :, :], in_=w1_ext[:, :])
                xk = []
                for k in range(4):
                    xt = apool.tile([128, NPADROWS], bfl, tag=f"xk{k}")
                    nc.sync.dma_start(out=xt[:, :],
                                      in_=xT_ext[k * 128:(k + 1) * 128, :])
                    xk.append(xt)
                for b in range(NB):
                    hps = apsum.tile([128, D1], f32, tag="hps")
                    for k in range(4):
                        nc.tensor.matmul(
                            hps[:, :], lhsT=xk[k][:, b * BLK:(b + 1) * BLK],
                            rhs=w1_t[:, k, :], start=(k == 0), stop=(k == 3))
                    nc.scalar.activation(out=tab1_sb[:, b, 0:D1], in_=hps[:, :],
                                         func=mybir.ActivationFunctionType.Copy)
                    tmp = apool.tile([128, D1], f32, tag="atmp")
                    nc.vector.tensor_tensor(out=tmp[:, :], in0=hps[:, :],
                                            in1=a1s_t[:, :],
                                            op=mybir.AluOpType.mult)
                    nc.vector.tensor_reduce(
                        out=tab1_sb[:, b, 64:80].bitcast(f32),
                        in_=tmp[:, :].rearrange("p (h c) -> p h c", h=H1, c=HID),
                        axis=mybir.AxisListType.X, op=mybir.AluOpType.add)
                    tmp2 = apool.tile([128, D1], f32, tag="atmp2")
                    nc.vector.tensor_tensor(out=tmp2[:, :], in0=hps[:, :],
                                            in1=a1d_t[:, :],
                                            op=mybir.AluOpType.mult)
                    nc.vector.tensor_reduce(
                        out=tab1_sb[:, b, 80:96].bitcast(f32),
                        in_=tmp2[:, :].rearrange("p (h c) -> p h c", h=H1, c=HID),
                        axis=mybir.AxisListType.X, op=mybir.AluOpType.add)

            _dma_table_out(nc, t1_shard, tab1_sb)
            nc.gpsimd.collective_compute(
                "AllGather", mybir.AluOpType.bypass, replica_groups=rg,
                ins=[t1_shard.ap().opt()], outs=[t1_full.ap().opt()])

            if stage >= 2:
                _edge_phase(nc, tc, layer=1, table_full=t1_full,
                            idx_ext=idx1_ext, msk_ext=msk1_ext, ident=ident,
                            plan=plan1, a_s=a2s_t, a_d=a2d_t, bias=b1_t,
                            w2_t=w2_t, tab_out=tab2_sb, out_ext=None,
                            b2_t=None, dbg_ext=dbg1_ext)

                _dma_table_out(nc, t2_shard, tab2_sb)
                nc.gpsimd.collective_compute(
                    "AllGather", mybir.AluOpType.bypass, replica_groups=rg,
                    ins=[t2_shard.ap().opt()], outs=[t2_full.ap().opt()])

            if stage >= 3:
                _edge_phase(nc, tc, layer=2, table_full=t2_full,
                            idx_ext=idx2_ext, msk_ext=msk2_ext, ident=ident,
                            plan=plan2, a_s=None, a_d=None, bias=None,
                            w2_t=None, tab_out=None, out_ext=out_ext,
                            b2_t=b2_t, dbg_ext=None)
            else:
                with tc.tile_pool(name="stub", bufs=1) as spool:
                    z = spool.tile([128, C2], f32)
                    nc.vector.memset(z[:, :], 0.0)
                    for b in range(NB):
                        nc.sync.dma_start(out=out_ext[b * BLK:(b + 1) * BLK, :],
                                          in_=z[:, :])

    nc.compile()
    return nc


def _dma_table_out(nc, bounce, tab_sb):
    full = NB - 1
    rows = SHARD - full * BLK  # 106
    nc.sync.dma_start(
        out=bounce[0:full * BLK, :].rearrange("(b p) c -> p b c", p=BLK, b=full),
        in_=tab_sb[:, 0:full, :])
    nc.sync.dma_start(out=bounce[full * BLK:SHARD, :], in_=tab_sb[0:rows, full, :])


def _edge_phase(nc, tc, layer, table_full, idx_ext, msk_ext, ident,
                plan, a_s, a_d, bias, w2_t, tab_out, out_ext, b2_t,
                dbg_ext=None):
    """ELL-layout per-edge softmax-aggregate phase (see module docstring)."""
    if layer == 1:
        NH, CH, CC = H1, HID, D1       # 8 heads x 8 ch = 64
        asrc_sl = (64, 80)
        adst_sl = (80, 96)
    else:
        NH, CH, CC = 1, C2, C2         # 1 head x 40
        asrc_sl = (64, 66)
        adst_sl = (66, 68)
    NCOL = CC + NH
    sub = int(os.environ.get("K_EDGE_SUB", "4"))

    with tc.tile_pool(name=f"e{layer}", bufs=2) as pool, \
         tc.tile_pool(name=f"e{layer}_ps", bufs=2, space="PSUM") as psum, \
         tc.tile_pool(name=f"e{layer}_ps2", bufs=2, space="PSUM") as psum2:
        for blocks, goff, gch, nlo_ch in plan.groups:
            g_t = pool.tile([128, gch, TROW], bfl, tag="gath")
            nhi_ch = gch - nlo_ch
            idxg = pool.tile([128, gch * 8], i16, tag="idxg")
            nc.sync.dma_start(out=idxg[:, :],
                              in_=idx_ext[:, goff * 8:(goff + gch) * 8])
            mskg = pool.tile([128, gch], bfl, tag="mskg")
            nc.sync.dma_start(out=mskg[:, :], in_=msk_ext[:, goff:goff + gch])
            nc.gpsimd.dma_gather(
                out_ap=g_t[:, 0:nlo_ch, :], in_ap=table_full[0:SPLIT, :],
                idxs_ap=idxg[:, 0:nlo_ch * 8],
                num_idxs=nlo_ch * BLK, num_idxs_reg=nlo_ch * BLK,
                elem_size=TROW, single_packet=False)
            nc.gpsimd.dma_gather(
                out_ap=g_t[:, nlo_ch:gch, :], in_ap=table_full[SPLIT:N_NODES, :],
                idxs_ap=idxg[:, nlo_ch * 8:gch * 8],
                num_idxs=nhi_ch * BLK, num_idxs_reg=nhi_ch * BLK,
                elem_size=TROW, single_packet=False, queue_num=1)

            if sub < 2:
                continue
            for b in blocks:
                lo0, nbl = plan.lo_runs[b]
                hi0, nbh = plan.hi_runs[b]
                lo0 -= goff
                hi0 -= goff
                mlo = plan.mlo_ch[b] - goff
                mhi = plan.mhi_ch[b] - goff

                # a_dst[dst] per partition: blend member-lo / member-hi rows
                mf = pool.tile([128, 2], f32, tag="memmask")
                nc.vector.tensor_copy(out=mf[:, 0:1], in_=mskg[:, mlo:mlo + 1])
                nc.vector.tensor_copy(out=mf[:, 1:2], in_=mskg[:, mhi:mhi + 1])
                adst = pool.tile([128, NH], f32, tag="adst")
                tmp_hi = pool.tile([128, NH], f32, tag="adsth")
                nc.vector.tensor_scalar(
                    out=tmp_hi[:, :],
                    in0=g_t[:, mhi, adst_sl[0]:adst_sl[1]].bitcast(f32),
                    scalar1=mf[:, 1:2], scalar2=None,
                    op0=mybir.AluOpType.mult)
                nc.vector.scalar_tensor_tensor(
                    out=adst[:, :],
                    in0=g_t[:, mlo, adst_sl[0]:adst_sl[1]].bitcast(f32),
                    scalar=mf[:, 0:1], in1=tmp_hi[:, :],
                    op0=mybir.AluOpType.mult, op1=mybir.AluOpType.add)

                ps = psum.tile([128, NCOL], f32, tag="agg")
                r_ts = []
                for r0, rn in ((lo0, nbl), (hi0, nbh)):
                    tg = "h" if r0 == hi0 else "l"
                    e_t = pool.tile([128, rn, NH], f32, tag=f"elog{tg}")
                    nc.vector.tensor_tensor(
                        out=e_t[:, :, :],
                        in0=g_t[:, r0:r0 + rn,
                                asrc_sl[0]:asrc_sl[1]].bitcast(f32),
                        in1=adst[:, None, :].to_broadcast([128, rn, NH]),
                        op=mybir.AluOpType.add)
                    lr_t = pool.tile([128, rn, NH], f32, tag=f"lr{tg}")
                    nc.vector.scalar_tensor_tensor(
                        out=lr_t[:, :, :], in0=e_t[:, :, :], scalar=NEG_SLOPE,
                        in1=e_t[:, :, :], op0=mybir.AluOpType.mult,
                        op1=mybir.AluOpType.max)
                    exf = pool.tile([128, rn, NH], f32, tag=f"exf{tg}")
                    nc.scalar.activation(out=exf[:, :, :], in_=lr_t[:, :, :],
                                         func=mybir.ActivationFunctionType.Exp)
                    r_t = pool.tile([128, rn, NCOL], bfl, tag=f"rmat{tg}")
                    nc.vector.tensor_tensor(
                        out=r_t[:, :, CC:NCOL], in0=exf[:, :, :],
                        in1=mskg[:, r0:r0 + rn, None].to_broadcast([128, rn, NH]),
                        op=mybir.AluOpType.mult)
                    nc.vector.tensor_tensor(
                        out=r_t[:, :, 0:CC].rearrange("p g (h c) -> p g h c",
                                                      h=NH, c=CH),
                        in0=g_t[:, r0:r0 + rn, 0:CC].rearrange(
                            "p g (h c) -> p g h c", h=NH, c=CH),
                        in1=r_t[:, :, CC:NCOL, None].to_broadcast(
                            [128, rn, NH, CH]),
                        op=mybir.AluOpType.mult)
                    r_ts.append((r_t, rn))
                if sub < 3:
                    continue
                nmm = sum(rn for _, rn in r_ts)
                ji = 0
                for r_t, rn in r_ts:
                    for j in range(rn):
                        nc.tensor.matmul(ps[:, :], lhsT=ident[:, :],
                                         rhs=r_t[:, j, :], start=(ji == 0),
                                         stop=(ji == nmm - 1))
                        ji += 1

                den = pool.tile([128, NH], f32, tag="den")
                nc.vector.tensor_scalar(out=den[:, :], in0=ps[:, CC:NCOL],
                                        scalar1=1e-16, scalar2=None,
                                        op0=mybir.AluOpType.add)
                recip = pool.tile([128, NH], f32, tag="recip")
                nc.vector.reciprocal(out=recip[:, :], in_=den[:, :])
                o_t = pool.tile([128, CC], f32, tag="outb")
                nc.vector.tensor_tensor(
                    out=o_t[:, :].rearrange("p (h c) -> p h c", h=NH, c=CH),
                    in0=ps[:, 0:CC].rearrange("p (h c) -> p h c", h=NH, c=CH),
                    in1=recip[:, :, None].to_broadcast([128, NH, CH]),
                    op=mybir.AluOpType.mult)

                if sub < 4:
                    continue
                if layer == 1:
                    obt = pool.tile([128, CC], f32, tag="outbt")
                    nc.vector.tensor_tensor(out=obt[:, :], in0=o_t[:, :],
                                            in1=bias[:, :],
                                            op=mybir.AluOpType.add)
                    ob = pool.tile([128, CC], bfl, tag="outbf")
                    nc.vector.tensor_scalar(out=ob[:, :], in0=obt[:, :],
                                            scalar1=0.0, scalar2=None,
                                            op0=mybir.AluOpType.max)
                    if dbg_ext is not None:
                        dtt = pool.tile([128, NCOL], f32, tag="dbgt")
                        nc.vector.tensor_copy(out=dtt[:, :], in_=ps[:, :])
                        nc.sync.dma_start(
                            out=dbg_ext[b * BLK:(b + 1) * BLK, :],
                            in_=dtt[:, :])
                    tps = psum2.tile([D1, 128], bfl, tag="tp")
                    nc.tensor.transpose(tps[:, :], ob[:, :], ident[:, :])
                    h1T = pool.tile([D1, 128], bfl, tag="h1T")
                    nc.vector.tensor_copy(out=h1T[:, :], in_=tps[:, :])
                    h2ps = psum2.tile([128, C2], f32, tag="h2")
                    nc.tensor.matmul(h2ps[:, :], lhsT=h1T[:, :], rhs=w2_t[:, :],
                                     start=True, stop=True)
                    nc.scalar.activation(out=tab_out[:, b, 0:C2],
                                         in_=h2ps[:, :],
                                         func=mybir.ActivationFunctionType.Copy)
                    t1 = pool.tile([128, C2], f32, tag="t1")
                    nc.vector.tensor_tensor(out=t1[:, :], in0=h2ps[:, :],
                                            in1=a_s[:, :],
                                            op=mybir.AluOpType.mult)
                    nc.vector.tensor_reduce(
                        out=tab_out[:, b, 64:66].bitcast(f32), in_=t1[:, :],
                        axis=mybir.AxisListType.X, op=mybir.AluOpType.add)
                    t2 = pool.tile([128, C2], f32, tag="t2")
                    nc.vector.tensor_tensor(out=t2[:, :], in0=h2ps[:, :],
                                            in1=a_d[:, :],
                                            op=mybir.AluOpType.mult)
                    nc.vector.tensor_reduce(
                        out=tab_out[:, b, 66:68].bitcast(f32), in_=t2[:, :],
                        axis=mybir.AxisListType.X, op=mybir.AluOpType.add)
                else:
                    lg = pool.tile([128, C2], f32, tag="logits")
                    nc.vector.tensor_tensor(out=lg[:, :], in0=o_t[:, :],
                                            in1=b2_t[:, :],
                                            op=mybir.AluOpType.add)
                    negm = pool.tile([128, 1], f32, tag="negm")
                    nc.vector.tensor_reduce(out=negm[:, :], in_=lg[:, :],
                                            axis=mybir.AxisListType.X,
                                            op=mybir.AluOpType.max, negate=True)
                    ex = pool.tile([128, C2], f32, tag="sfex")
                    ssum = pool.tile([128, 1], f32, tag="ssum")
                    nc.scalar.activation(out=ex[:, :], in_=lg[:, :],
                                         func=mybir.ActivationFunctionType.Exp,
                                         bias=negm[:, :], accum_out=ssum[:, :])
                    lse = pool.tile([128, 1], f32, tag="lse")
                    nc.scalar.activation(out=lse[:, :], in_=ssum[:, :],
                                         func=mybir.ActivationFunctionType.Ln)
                    res = pool.tile([128, C2], f32, tag="res")
                    nc.vector.scalar_tensor_tensor(
                        out=res[:, :], in0=lg[:, :], scalar=negm[:, :],
                        in1=lse[:, :].to_broadcast([128, C2]),
                        op0=mybir.AluOpType.add, op1=mybir.AluOpType.subtract)
                    nc.sync.dma_start(out=out_ext[b * BLK:(b + 1) * BLK, :],
                                      in_=res[:, :])


def _host_inputs(x, W1, att_src1, att_dst1, b1, W2, att_src2, att_dst2, b2,
                 plan1, plan2):
    NPADROWS = NB * BLK
    w1r = np.ascontiguousarray(
        np.asarray(W1, np.float32).reshape(4, 128, D1).transpose(1, 0, 2)
    ).reshape(128, 4 * D1).astype(bf16)
    rep = lambda v, n: np.tile(np.asarray(v, np.float32).reshape(1, n),
                               (128, 1)).astype(np.float32)
    x32 = np.asarray(x, np.float32)

    in_maps = []
    for c in range(N_CORES):
        xs = x32[c * SHARD:(c + 1) * SHARD]
        xT = np.zeros((F_IN, NPADROWS), bf16)
        xT[:, :SHARD] = xs.T.astype(bf16)
        in_maps.append({
            "xT": xT,
            "w1r": w1r,
            "w2": np.asarray(W2, np.float32).astype(bf16),
            "a1srep": rep(att_src1, D1),
            "a1drep": rep(att_dst1, D1),
            "a2srep": rep(att_src2, C2),
            "a2drep": rep(att_dst2, C2),
            "b1rep": rep(b1, D1),
            "b2rep": rep(b2, C2),
            "idx1": plan1.idx_streams[c],
            "msk1": plan1.mask_streams[c],
            "idx2": plan2.idx_streams[c],
            "msk2": plan2.mask_streams[c],
        })
    return in_maps


def kernel_run(inputs, trace=False):
    """Build (cached), run, and return (out [50000,40] f32, exec_time_ns)."""
    edge_index = inputs["edge_index"]
    plan1, plan2 = _prep(edge_index)

    key = (tuple(plan1.nch_lo), tuple(plan1.nch_hi),
           tuple(plan2.nch_lo), tuple(plan2.nch_hi))
    if key not in _CACHE:
        _CACHE[key] = _build(plan1, plan2)
    nc = _CACHE[key]

    in_maps = _host_inputs(
        inputs["x"], inputs["W1"], inputs["att_src1"], inputs["att_dst1"],
        inputs["b1"], inputs["W2"], inputs["att_src2"], inputs["att_dst2"],
        inputs["b2"], plan1, plan2)

    if trace:
        _install_ntff_hook()
    res = run_bass_kernel_spmd(nc, in_maps, core_ids=list(range(N_CORES)),
                               trace=trace)
    # undo the block permutation (output rows are layer-2 block slots)
    out = np.zeros((N_NODES, C2), np.float32)
    for c in range(N_CORES):
        o = res.results[c]["out"]
        mem = plan2.perm[c]
        valid = mem >= 0
        out[c * SHARD + mem[valid]] = o[valid]
    return out, res.exec_time_ns


def kernel(**inputs):
    out, _ = kernel_run(inputs)
    return out



# revision 5
# speedup vs baseline: 2.2595x; 2.2595x over previous
"""GAT 2-layer GNN kernel for 8 Trainium2 NeuronCores (v2).

Strategy (graph/data parallel, per the sharding hint):
  - Nodes are assigned to (core, slot) by global in-degree sort dealt
    round-robin across the 8 cores, then within-core block packing by
    (lo-degree, hi-degree) so each ELL block of 128 dst slots has a tight
    max degree across ALL cores (the SPMD program shares chunk counts).
  - One unified slot-ordered row map serves BOTH layers: table row of a
    node = core*6272 + slot. The int16 gather-index split is at the core
    4 boundary (row 25088), so each half has 25088 rows < 32768 and the
    lo/hi membership of an edge depends only on its src's core.
  - Per layer, each core builds its shard's table rows [h | a_src | pad]
    (a_dst stays resident in SBUF - it is only needed for the core's own
    dst slots), an AllGather replicates the full table, and per-edge rows
    are fetched with dma_gather in ELL layout (partition = dst slot in
    block, chunk = position in the dst's edge list). Padding slots point
    at a dedicated pad row whose a_src = -30000, so exp(leakyrelu(e)) = 0
    kills them with no mask stream.
  - Gathers are issued 2 groups ahead across rotating SWDGE queues so
    descriptor generation + DMA drain overlap the softmax-aggregate
    compute of earlier groups.
  - The segment softmax-aggregate per block: R[p, chunk, :] =
    [w*h | w] with w = exp(leakyrelu(a_src[src]+a_dst[dst])), summed over
    chunks by an identity-weighted PSUM matmul chain, giving
    [numerator | denominator] per dst slot. Layer 1 tail computes
    relu(out+b1) @ W2 and the layer-2 table rows; layer 2 tail fuses
    bias + log_softmax. The host undoes the slot permutation.
"""

import os
import sys

sys.path.insert(0, "/opt/trn_rl_repo")

import numpy as np
import ml_dtypes

import concourse.bacc as bacc
import concourse.mybir as mybir
from concourse import tile
from concourse.bass_utils import run_bass_kernel_spmd
from concourse.masks import make_identity

bf16 = ml_dtypes.bfloat16

N_NODES = 50000
F_IN = 512
H1 = 8
HID = 8
D1 = H1 * HID  # 64
C2 = 40
N_CORES = 8
SHARD = N_NODES // N_CORES  # 6250
BLK = 128
NB = (SHARD + BLK - 1) // BLK  # 49
SLOTS = NB * BLK  # 6272 (last 22 scratch)
HALF = 4 * SLOTS  # 25088 rows per int16 gather half
PAD_IDX = SLOTS - 1  # scratch row 6271 of core 0 (lo) / core 4 (hi)
NEG_SLOPE = 0.2
NEGV = -30000.0
TROW = 128  # table row: 128 bf16 = 256 bytes

NQUEUES = int(os.environ.get("K_QUEUES", "4"))
PF = int(os.environ.get("K_PF", "2"))
TARGET_GCH = int(os.environ.get("K_TGCH", "110"))

f32 = mybir.dt.float32
bfl = mybir.dt.bfloat16
i16 = mybir.dt.int16

_CACHE = {}


def _install_ntff_hook():
    """Provide antenv.axon_hooks if the image lacks it, driving NTFF
    profiling via the injected libaxon_pjrt.so C ABI (see trn_boot)."""
    try:
        from antenv.axon_hooks import get_axon_ntff_profile_hook  # noqa: F401
        return
    except ImportError:
        pass
    import contextlib
    import ctypes
    import types

    so_path = "/opt/axon/libaxon_pjrt.so"
    try:
        lib = ctypes.CDLL(so_path)
    except OSError:
        return
    if not hasattr(lib, "axon_start_nrt_profile"):
        return
    lib.axon_start_nrt_profile.argtypes = [ctypes.POINTER(ctypes.c_int64),
                                           ctypes.c_size_t]
    lib.axon_start_nrt_profile.restype = ctypes.c_int64
    lib.axon_stop_nrt_profile.argtypes = [ctypes.c_char_p]
    lib.axon_stop_nrt_profile.restype = ctypes.c_int64

    @contextlib.contextmanager
    def _hook(output_dir, device_ids):
        import jax
        jax.devices()
        if device_ids:
            ids = (ctypes.c_int64 * len(device_ids))(*device_ids)
            rc = lib.axon_start_nrt_profile(ids, len(device_ids))
        else:
            rc = lib.axon_start_nrt_profile(None, 0)
        if rc != 0:
            raise RuntimeError(f"axon_start_nrt_profile rc={rc}")
        try:
            yield
        finally:
            n = lib.axon_stop_nrt_profile(str(output_dir).encode())
            print(f"ntff profile: {n} file(s) written to {output_dir}")

    import antenv
    mod = types.ModuleType("antenv.axon_hooks")
    mod.get_axon_ntff_profile_hook = lambda: _hook
    mod.set_axon_ntff_profile_hook = lambda h: None
    sys.modules["antenv.axon_hooks"] = mod
    antenv.axon_hooks = mod


def _ceil(a, b):
    return (a + b - 1) // b


def _running_count(k):
    """pos[i] = number of j<i with k[j]==k[i]; k is sorted."""
    n = len(k)
    if n == 0:
        return np.zeros(0, np.int64)
    starts = np.r_[0, np.flatnonzero(np.diff(k)) + 1]
    run_id = np.zeros(n, np.int64)
    run_id[starts[1:]] = 1
    run_id = np.cumsum(run_id)
    return np.arange(n) - starts[run_id]


class Plan:
    pass


def _prep(edge_index):
    src0 = np.asarray(edge_index[0], dtype=np.int64)
    dst0 = np.asarray(edge_index[1], dtype=np.int64)
    loops = np.arange(N_NODES, dtype=np.int64)
    src = np.concatenate([src0, loops])
    dst = np.concatenate([dst0, loops])

    # --- lo/hi group by in-degree rank parity (stable under later
    #     core/slot shuffles within a group) ---
    k_in = np.bincount(dst, minlength=N_NODES)
    order = np.argsort(-k_in, kind="stable")
    rank = np.empty(N_NODES, np.int64)
    rank[order] = np.arange(N_NODES)
    lo_node = (rank % 2) == 0  # 25000 nodes -> cores 0-3

    klo = np.bincount(dst, weights=lo_node[src].astype(np.float64),
                      minlength=N_NODES).astype(np.int64)
    khi = k_in - klo

    # --- core+slot: sort each group, deal round-robin to its 4 cores so
    #     all 4 share near-identical per-block degree profiles ---
    grp_nodes = [np.flatnonzero(lo_node), np.flatnonzero(~lo_node)]

    def eval_key(strata_blocks, swap):
        """Sort each group by primary key desc in strata of
        strata_blocks*4*BLK nodes, secondary key desc within stratum."""
        k1, k2 = (khi, klo) if swap else (klo, khi)
        core_of = np.zeros(N_NODES, np.int64)
        slot_of = np.zeros(N_NODES, np.int64)
        ssz = strata_blocks * 4 * BLK
        for g in range(2):
            nodes = grp_nodes[g]
            o = nodes[np.argsort(-k1[nodes], kind="stable")]
            strat = np.arange(len(o)) // ssz
            o = o[np.lexsort((-k2[o], strat))]
            q = np.arange(len(o))
            core_of[o] = g * 4 + q % 4
            slot_of[o] = q // 4
        KL = np.zeros((N_CORES, SLOTS), np.int64)
        KH = np.zeros((N_CORES, SLOTS), np.int64)
        KL[core_of, slot_of] = klo
        KH[core_of, slot_of] = khi
        nl = np.maximum(KL.reshape(N_CORES, NB, BLK).max(axis=(0, 2)), 1)
        nh = np.maximum(KH.reshape(N_CORES, NB, BLK).max(axis=(0, 2)), 1)
        return core_of, slot_of, nl, nh

    best = None
    for swap in (False, True):
        for sb in (1, 2, 3, 5, 7, 10, 49):
            core_of, slot_of, nl, nh = eval_key(sb, swap)
            tot = int((nl + nh).sum())
            if best is None or tot < best[0]:
                best = (tot, core_of, slot_of, nl, nh)
    _, core_of, slot_of, nch_lo, nch_hi = best

    node_of = np.full((N_CORES, SLOTS), -1, np.int64)
    node_of[core_of, slot_of] = np.arange(N_NODES)

    rows = core_of * SLOTS + slot_of
    hi_edge = (core_of[src] >= 4)
    idxval = rows[src] - hi_edge * HALF  # 0..25087

    # --- group packing: balanced bins of blocks ---
    w = nch_lo + nch_hi
    G = max(1, _ceil(int(w.sum()), TARGET_GCH))
    bins = [[] for _ in range(G)]
    sums = np.zeros(G, np.int64)
    for b in np.argsort(-w, kind="stable"):
        g = int(np.argmin(sums))
        bins[g].append(int(b))
        sums[g] += w[b]

    lo_off = np.zeros(NB, np.int64)
    hi_off = np.zeros(NB, np.int64)
    groups = []
    goff = 0
    for blocks in bins:
        ch = goff
        for b in blocks:
            lo_off[b] = ch
            ch += int(nch_lo[b])
        nlo = ch - goff
        for b in blocks:
            hi_off[b] = ch
            ch += int(nch_hi[b])
        groups.append((blocks, goff, ch - goff, nlo))
        goff = ch
    total_ch = goff

    # --- per-core ELL index streams (shared by both layers) ---
    e_core = core_of[dst]
    e_slot = slot_of[dst]
    idx_streams = []
    for c in range(N_CORES):
        sel = e_core == c
        s_slot = e_slot[sel]
        s_hi = hi_edge[sel].astype(np.int64)
        s_idx = idxval[sel]
        s_blk = s_slot // BLK
        s_p = s_slot % BLK
        key = s_slot * 2 + s_hi
        o = np.argsort(key, kind="stable")
        inv = np.empty_like(o)
        inv[o] = np.arange(len(o))
        pos = _running_count(key[o])[inv]
        ch = np.where(s_hi == 1, hi_off[s_blk], lo_off[s_blk]) + pos
        flat = ch * BLK + s_p
        idx = np.full(total_ch * BLK, PAD_IDX, np.int16)
        idx[flat] = s_idx.astype(np.int16)
        idx_w = np.tile(idx.reshape(total_ch * 8, 16).T, (8, 1)).copy()
        idx_streams.append(idx_w)

    plan = Plan()
    plan.core_of = core_of
    plan.slot_of = slot_of
    plan.node_of = node_of
    plan.nch_lo = nch_lo
    plan.nch_hi = nch_hi
    plan.lo_off = lo_off
    plan.hi_off = hi_off
    plan.groups = groups
    plan.total_ch = total_ch
    plan.idx_streams = idx_streams
    return plan


def _build(plan):
    nc = bacc.Bacc("TRN2", target_bir_lowering=False, debug=False,
                   num_devices=N_CORES, num_swdge_queues=NQUEUES)

    xT_ext = nc.declare_dram_parameter("xT", [F_IN, SLOTS], bfl, isOutput=False)
    w1_ext = nc.declare_dram_parameter("w1r", [128, 4 * D1], bfl, isOutput=False)
    w2_ext = nc.declare_dram_parameter("w2", [D1, C2], bfl, isOutput=False)
    a1s_ext = nc.declare_dram_parameter("a1srep", [128, D1], f32, isOutput=False)
    a1d_ext = nc.declare_dram_parameter("a1drep", [128, D1], f32, isOutput=False)
    a2s_ext = nc.declare_dram_parameter("a2srep", [128, C2], f32, isOutput=False)
    a2d_ext = nc.declare_dram_parameter("a2drep", [128, C2], f32, isOutput=False)
    b1_ext = nc.declare_dram_parameter("b1rep", [128, D1], f32, isOutput=False)
    b2_ext = nc.declare_dram_parameter("b2rep", [128, C2], f32, isOutput=False)
    idx_ext = nc.declare_dram_parameter("idx", [128, plan.total_ch * 8], i16,
                                        isOutput=False)
    out_ext = nc.declare_dram_parameter("out", [SLOTS, C2], f32, isOutput=True)

    t1_shard = nc.dram_tensor("t1_shard", [SLOTS, TROW], bfl)
    t1_full = nc.dram_tensor("t1_full", [N_CORES * SLOTS, TROW], bfl,
                             addr_space="Shared")
    t2_shard = nc.dram_tensor("t2_shard", [SLOTS, TROW], bfl)
    t2_full = nc.dram_tensor("t2_full", [N_CORES * SLOTS, TROW], bfl,
                             addr_space="Shared")

    rg = [list(range(N_CORES))]

    with tile.TileContext(nc) as tc:
        with tc.tile_pool(name="const", bufs=1) as cpool:
            ident = cpool.tile([128, 128], bfl)
            make_identity(nc, ident[:, :])
            a1s_t = cpool.tile([128, D1], f32)
            nc.sync.dma_start(out=a1s_t[:, :], in_=a1s_ext[:, :])
            a1d_t = cpool.tile([128, D1], f32)
            nc.sync.dma_start(out=a1d_t[:, :], in_=a1d_ext[:, :])
            a2s_t = cpool.tile([128, C2], f32)
            nc.sync.dma_start(out=a2s_t[:, :], in_=a2s_ext[:, :])
            a2d_t = cpool.tile([128, C2], f32)
            nc.sync.dma_start(out=a2d_t[:, :], in_=a2d_ext[:, :])
            b1_t = cpool.tile([128, D1], f32)
            nc.sync.dma_start(out=b1_t[:, :], in_=b1_ext[:, :])
            b2_t = cpool.tile([128, C2], f32)
            nc.sync.dma_start(out=b2_t[:, :], in_=b2_ext[:, :])
            w2_t = cpool.tile([D1, C2], bfl)
            nc.sync.dma_start(out=w2_t[:, :], in_=w2_ext[:, :])
            idx_t = cpool.tile([128, plan.total_ch * 8], i16)
            nc.sync.dma_start(out=idx_t[:, :], in_=idx_ext[:, :])
            tab1 = cpool.tile([128, NB, TROW], bfl)
            tab2 = cpool.tile([128, NB, TROW], bfl)
            ad1 = cpool.tile([128, NB, H1], f32)
            ad2 = cpool.tile([128, NB, 1], f32)

            # ---------------- Phase A: h1 = x @ W1 + attention scalars ----
            with tc.tile_pool(name="phA", bufs=2) as apool, \
                 tc.tile_pool(name="phA_ps", bufs=2, space="PSUM") as apsum:
                w1_t = apool.tile([128, 4, D1], bfl, tag="w1")
                nc.sync.dma_start(out=w1_t[:, :, :], in_=w1_ext[:, :])
                xk = []
                for k in range(4):
                    xt = apool.tile([128, SLOTS], bfl, tag=f"xk{k}")
                    nc.sync.dma_start(out=xt[:, :],
                                      in_=xT_ext[k * 128:(k + 1) * 128, :])
                    xk.append(xt)
                for b in range(NB):
                    hps = apsum.tile([128, D1], f32, tag="hps")
                    for k in range(4):
                        nc.tensor.matmul(
                            hps[:, :], lhsT=xk[k][:, b * BLK:(b + 1) * BLK],
                            rhs=w1_t[:, k, :], start=(k == 0), stop=(k == 3))
                    nc.scalar.activation(out=tab1[:, b, 0:D1], in_=hps[:, :],
                                         func=mybir.ActivationFunctionType.Copy)
                    tmp = apool.tile([128, D1], f32, tag="atmp")
                    nc.vector.tensor_tensor(out=tmp[:, :], in0=hps[:, :],
                                            in1=a1s_t[:, :],
                                            op=mybir.AluOpType.mult)
                    nc.vector.tensor_reduce(
                        out=tab1[:, b, 64:80].bitcast(f32),
                        in_=tmp[:, :].rearrange("p (h c) -> p h c", h=H1, c=HID),
                        axis=mybir.AxisListType.X, op=mybir.AluOpType.add)
                    tmp2 = apool.tile([128, D1], f32, tag="atmp2")
                    nc.vector.tensor_tensor(out=tmp2[:, :], in0=hps[:, :],
                                            in1=a1d_t[:, :],
                                            op=mybir.AluOpType.mult)
                    nc.vector.tensor_reduce(
                        out=ad1[:, b, :],
                        in_=tmp2[:, :].rearrange("p (h c) -> p h c", h=H1, c=HID),
                        axis=mybir.AxisListType.X, op=mybir.AluOpType.add)
            # pad rows: a_src = NEGV so exp(leakyrelu(...)) == 0
            padv = cpool.tile([22, 8], f32)
            nc.vector.memset(padv[:, :], NEGV)

            nc.sync.dma_start(
                out=t1_shard[:, :].rearrange("(b p) c -> p b c", p=BLK, b=NB),
                in_=tab1[:, :, :])
            nc.sync.dma_start(out=t1_shard[SHARD:SLOTS, 64:80].bitcast(f32),
                              in_=padv[:, :])
            nc.gpsimd.collective_compute(
                "AllGather", mybir.AluOpType.bypass, replica_groups=rg,
                ins=[t1_shard.ap().opt()], outs=[t1_full.ap().opt()])

            _edge_phase(nc, tc, layer=1, table_full=t1_full, idx_t=idx_t,
                        ident=ident, plan=plan, adst_sb=ad1, bias=b1_t,
                        w2_t=w2_t, a2s_t=a2s_t, a2d_t=a2d_t, tab2=tab2,
                        ad2=ad2, b2_t=None, out_ext=None)

            nc.sync.dma_start(
                out=t2_shard[:, :].rearrange("(b p) c -> p b c", p=BLK, b=NB),
                in_=tab2[:, :, :])
            nc.sync.dma_start(out=t2_shard[SHARD:SLOTS, 40:42].bitcast(f32),
                              in_=padv[:, 0:1])
            nc.gpsimd.collective_compute(
                "AllGather", mybir.AluOpType.bypass, replica_groups=rg,
                ins=[t2_shard.ap().opt()], outs=[t2_full.ap().opt()])

            _edge_phase(nc, tc, layer=2, table_full=t2_full, idx_t=idx_t,
                        ident=ident, plan=plan, adst_sb=ad2, bias=None,
                        w2_t=None, a2s_t=None, a2d_t=None, tab2=None,
                        ad2=None, b2_t=b2_t, out_ext=out_ext)

    nc.compile()
    return nc


def _edge_phase(nc, tc, layer, table_full, idx_t, ident, plan, adst_sb,
                bias, w2_t, a2s_t, a2d_t, tab2, ad2, b2_t, out_ext):
    if layer == 1:
        NH, CH, CC = H1, HID, D1       # 8 heads x 8 ch = 64
        asrc_sl = (64, 80)
    else:
        NH, CH, CC = 1, C2, C2         # 1 head x 40
        asrc_sl = (40, 42)
    NCOL = CC + NH
    G = len(plan.groups)

    with tc.tile_pool(name=f"e{layer}g", bufs=PF + 1) as epool, \
         tc.tile_pool(name=f"e{layer}s", bufs=2) as pool, \
         tc.tile_pool(name=f"e{layer}_ps", bufs=2, space="PSUM") as psum, \
         tc.tile_pool(name=f"e{layer}_ps2", bufs=2, space="PSUM") as psum2:

        def issue(gi):
            blocks, goff, gch, nlo = plan.groups[gi]
            nhi = gch - nlo
            g_t = epool.tile([128, gch, TROW], bfl, tag="gath")
            nc.gpsimd.dma_gather(
                out_ap=g_t[:, 0:nlo, :], in_ap=table_full[0:HALF, :],
                idxs_ap=idx_t[:, goff * 8:(goff + nlo) * 8],
                num_idxs=nlo * BLK, num_idxs_reg=nlo * BLK,
                elem_size=TROW, single_packet=False,
                queue_num=(2 * gi) % NQUEUES)
            nc.gpsimd.dma_gather(
                out_ap=g_t[:, nlo:gch, :], in_ap=table_full[HALF:2 * HALF, :],
                idxs_ap=idx_t[:, (goff + nlo) * 8:(goff + gch) * 8],
                num_idxs=nhi * BLK, num_idxs_reg=nhi * BLK,
                elem_size=TROW, single_packet=False,
                queue_num=(2 * gi + 1) % NQUEUES)
            return g_t

        pending = []
        for gi in range(min(PF, G)):
            pending.append(issue(gi))

        for gi, (blocks, goff, gch, nlo) in enumerate(plan.groups):
            if gi + PF < G:
                pending.append(issue(gi + PF))
            g_t = pending.pop(0)

            for b in blocks:
                nbl = int(plan.nch_lo[b])
                nbh = int(plan.nch_hi[b])
                lo0 = int(plan.lo_off[b]) - goff
                hi0 = int(plan.hi_off[b]) - goff

                ps = psum.tile([128, NCOL], f32, tag="agg")
                r_ts = []
                for r0, rn, tg in ((lo0, nbl, "l"), (hi0, nbh, "h")):
                    e_t = pool.tile([128, rn, NH], f32, tag=f"elog{tg}")
                    nc.vector.tensor_tensor(
                        out=e_t[:, :, :],
                        in0=g_t[:, r0:r0 + rn,
                                asrc_sl[0]:asrc_sl[1]].bitcast(f32),
                        in1=adst_sb[:, b, None, :].to_broadcast([128, rn, NH]),
                        op=mybir.AluOpType.add)
                    lr_t = pool.tile([128, rn, NH], f32, tag=f"lr{tg}")
                    nc.vector.scalar_tensor_tensor(
                        out=lr_t[:, :, :], in0=e_t[:, :, :], scalar=NEG_SLOPE,
                        in1=e_t[:, :, :], op0=mybir.AluOpType.mult,
                        op1=mybir.AluOpType.max)
                    r_t = pool.tile([128, rn, NCOL], bfl, tag=f"rmat{tg}")
                    nc.scalar.activation(out=r_t[:, :, CC:NCOL],
                                         in_=lr_t[:, :, :],
                                         func=mybir.ActivationFunctionType.Exp)
                    nc.vector.tensor_tensor(
                        out=r_t[:, :, 0:CC].rearrange("p g (h c) -> p g h c",
                                                      h=NH, c=CH),
                        in0=g_t[:, r0:r0 + rn, 0:CC].rearrange(
                            "p g (h c) -> p g h c", h=NH, c=CH),
                        in1=r_t[:, :, CC:NCOL, None].to_broadcast(
                            [128, rn, NH, CH]),
                        op=mybir.AluOpType.mult)
                    r_ts.append((r_t, rn))

                nmm = nbl + nbh
                ji = 0
                for r_t, rn in r_ts:
                    for j in range(rn):
                        nc.tensor.matmul(ps[:, :], lhsT=ident[:, :],
                                         rhs=r_t[:, j, :], start=(ji == 0),
                                         stop=(ji == nmm - 1))
                        ji += 1

                den = pool.tile([128, NH], f32, tag="den")
                nc.vector.tensor_scalar(out=den[:, :], in0=ps[:, CC:NCOL],
                                        scalar1=1e-16, scalar2=None,
                                        op0=mybir.AluOpType.add)
                recip = pool.tile([128, NH], f32, tag="recip")
                nc.vector.reciprocal(out=recip[:, :], in_=den[:, :])
                o_t = pool.tile([128, CC], f32, tag="outb")
                nc.vector.tensor_tensor(
                    out=o_t[:, :].rearrange("p (h c) -> p h c", h=NH, c=CH),
                    in0=ps[:, 0:CC].rearrange("p (h c) -> p h c", h=NH, c=CH),
                    in1=recip[:, :, None].to_broadcast([128, NH, CH]),
                    op=mybir.AluOpType.mult)

                if layer == 1:
                    obt = pool.tile([128, CC], f32, tag="outbt")
                    nc.vector.tensor_tensor(out=obt[:, :], in0=o_t[:, :],
                                            in1=bias[:, :],
                                            op=mybir.AluOpType.add)
                    ob = pool.tile([128, CC], bfl, tag="outbf")
                    nc.vector.tensor_scalar(out=ob[:, :], in0=obt[:, :],
                                            scalar1=0.0, scalar2=None,
                                            op0=mybir.AluOpType.max)
                    tps = psum2.tile([D1, 128], bfl, tag="tp")
                    nc.tensor.transpose(tps[:, :], ob[:, :], ident[:, :])
                    h1T = pool.tile([D1, 128], bfl, tag="h1T")
                    nc.vector.tensor_copy(out=h1T[:, :], in_=tps[:, :])
                    h2ps = psum2.tile([128, C2], f32, tag="h2")
                    nc.tensor.matmul(h2ps[:, :], lhsT=h1T[:, :], rhs=w2_t[:, :],
                                     start=True, stop=True)
                    nc.scalar.activation(out=tab2[:, b, 0:C2],
                                         in_=h2ps[:, :],
                                         func=mybir.ActivationFunctionType.Copy)
                    t1 = pool.tile([128, C2], f32, tag="t1")
                    nc.vector.tensor_tensor(out=t1[:, :], in0=h2ps[:, :],
                                            in1=a2s_t[:, :],
                                            op=mybir.AluOpType.mult)
                    nc.vector.tensor_reduce(
                        out=tab2[:, b, 40:42].bitcast(f32), in_=t1[:, :],
                        axis=mybir.AxisListType.X, op=mybir.AluOpType.add)
                    t2 = pool.tile([128, C2], f32, tag="t2")
                    nc.vector.tensor_tensor(out=t2[:, :], in0=h2ps[:, :],
                                            in1=a2d_t[:, :],
                                            op=mybir.AluOpType.mult)
                    nc.vector.tensor_reduce(
                        out=ad2[:, b, :], in_=t2[:, :],
                        axis=mybir.AxisListType.X, op=mybir.AluOpType.add)
                else:
                    lg = pool.tile([128, C2], f32, tag="logits")
                    nc.vector.tensor_tensor(out=lg[:, :], in0=o_t[:, :],
                                            in1=b2_t[:, :],
                                            op=mybir.AluOpType.add)
                    negm = pool.tile([128, 1], f32, tag="negm")
                    nc.vector.tensor_reduce(out=negm[:, :], in_=lg[:, :],
                                            axis=mybir.AxisListType.X,
                                            op=mybir.AluOpType.max, negate=True)
                    ex = pool.tile([128, C2], f32, tag="sfex")
                    ssum = pool.tile([128, 1], f32, tag="ssum")
                    nc.scalar.activation(out=ex[:, :], in_=lg[:, :],
                                         func=mybir.ActivationFunctionType.Exp,
                                         bias=negm[:, :], accum_out=ssum[:, :])
                    lse = pool.tile([128, 1], f32, tag="lse")
                    nc.scalar.activation(out=lse[:, :], in_=ssum[:, :],
                                         func=mybir.ActivationFunctionType.Ln)
                    res = pool.tile([128, C2], f32, tag="res")
                    nc.vector.scalar_tensor_tensor(
                        out=res[:, :], in0=lg[:, :], scalar=negm[:, :],
                        in1=lse[:, :].to_broadcast([128, C2]),
                        op0=mybir.AluOpType.add, op1=mybir.AluOpType.subtract)
                    nc.sync.dma_start(out=out_ext[b * BLK:(b + 1) * BLK, :],
                                      in_=res[:, :])


def _host_inputs(x, W1, att_src1, att_dst1, b1, W2, att_src2, att_dst2, b2,
                 plan):
    w1r = np.ascontiguousarray(
        np.asarray(W1, np.float32).reshape(4, 128, D1).transpose(1, 0, 2)
    ).reshape(128, 4 * D1).astype(bf16)
    rep = lambda v, n: np.tile(np.asarray(v, np.float32).reshape(1, n),
                               (128, 1)).astype(np.float32)
    x32 = np.asarray(x, np.float32)

    in_maps = []
    for c in range(N_CORES):
        nodes = plan.node_of[c, :SHARD]
        xT = np.zeros((F_IN, SLOTS), bf16)
        xT[:, :SHARD] = x32[nodes].T.astype(bf16)
        in_maps.append({
            "xT": xT,
            "w1r": w1r,
            "w2": np.asarray(W2, np.float32).astype(bf16),
            "a1srep": rep(att_src1, D1),
            "a1drep": rep(att_dst1, D1),
            "a2srep": rep(att_src2, C2),
            "a2drep": rep(att_dst2, C2),
            "b1rep": rep(b1, D1),
            "b2rep": rep(b2, C2),
            "idx": plan.idx_streams[c],
        })
    return in_maps


def kernel_run(inputs, trace=False):
    """Build (cached), run, and return (out [50000,40] f32, exec_time_ns)."""
    edge_index = inputs["edge_index"]
    plan = _prep(edge_index)

    key = (tuple(plan.nch_lo), tuple(plan.nch_hi))
    if key not in _CACHE:
        _CACHE[key] = _build(plan)
    nc = _CACHE[key]

    in_maps = _host_inputs(
        inputs["x"], inputs["W1"], inputs["att_src1"], inputs["att_dst1"],
        inputs["b1"], inputs["W2"], inputs["att_src2"], inputs["att_dst2"],
        inputs["b2"], plan)

    if trace:
        _install_ntff_hook()
    res = run_bass_kernel_spmd(nc, in_maps, core_ids=list(range(N_CORES)),
                               trace=trace)
    out = np.zeros((N_NODES, C2), np.float32)
    for c in range(N_CORES):
        o = res.results[c]["out"]
        nodes = plan.node_of[c, :SHARD]
        out[nodes] = o[:SHARD]
    return out, res.exec_time_ns


def kernel(**inputs):
    out, _ = kernel_run(inputs)
    return out
